# revision 1
# baseline (speedup 1.0000x reference)
"""MHA block (10 heads, N=4096, D=640) on 8 trn2 NeuronCores.

Strategy: shard x by rows (512/core). The reference's raw reshapes make head
blocks contiguous in the flat Q/K/V buffers, so core c's query chunk-rows
[5120c, 5120(c+1)) are exactly Q rows [512c, 512(c+1)) and its attention
outputs are exactly output rows [512c, 512(c+1)). Only K and V need an
AllGather (bf16). Scores are computed transposed (keys on partitions) so both
attention matmuls take natural layouts; softmax denominator rides along as a
ones-column in the V matmul; exp runs on ACT straight out of PSUM. Per-core
head selection (10 heads across 8 cores) uses partition_id register arithmetic
with dynamic-offset DMA.
"""

import sys

sys.path.insert(0, "/opt/trn_rl_repo")

import numpy as np
import ml_dtypes

import concourse.bass as bass
import concourse.mybir as mybir
import concourse.tile as tile
from concourse import bacc
from concourse.bass_utils import run_bass_kernel_spmd
from concourse.masks import make_identity

BF16 = mybir.dt.bfloat16
F32 = mybir.dt.float32

NCORES = 8
N, D = 4096, 640
H, DK = 10, 64
NR = N // NCORES          # 512 x-rows per core
R = NR * (D // DK)        # 5120 chunk rows per core
NSB = 5                   # super-blocks of 1024 chunk rows
SCALE = 1.0 / 64.0        # 1/sqrt(N)
LN_EPS = 1e-5


def build_nc():
    nc = bacc.Bacc("TRN2", target_bir_lowering=False, debug=False,
                   num_devices=NCORES)

    xT_ext = nc.dram_tensor("xT", [D, NR], BF16, kind="ExternalInput")
    xf_ext = nc.dram_tensor("xf", [NR, D], F32, kind="ExternalInput")
    wqT_ext = nc.dram_tensor("wqT", [D, D], BF16, kind="ExternalInput")
    wkT_ext = nc.dram_tensor("wkT", [D, D], BF16, kind="ExternalInput")
    wvT_ext = nc.dram_tensor("wvT", [D, D], BF16, kind="ExternalInput")
    gamma_ext = nc.dram_tensor("gamma", [D], F32, kind="ExternalInput")
    beta_ext = nc.dram_tensor("beta", [D], F32, kind="ExternalInput")
    out_ext = nc.dram_tensor("out", [NR, D], F32, kind="ExternalOutput")

    with tile.TileContext(nc) as tc:
        with tc.tile_pool(name="singles", bufs=1) as singles, \
             tc.tile_pool(name="dram", bufs=1, space="DRAM") as dram:
            # ---- persistent SBUF state ----
            wq_sb = singles.tile([128, 5, D], BF16, name="wq_sb")
            wk_sb = singles.tile([128, 5, D], BF16, name="wk_sb")
            wv_sb = singles.tile([128, 5, D], BF16, name="wv_sb")
            xT_sb = singles.tile([128, 5, NR], BF16, name="xT_sb")
            qt_sb = singles.tile([128, R], BF16, name="qt_sb")  # Q^T chunk layout, duplicated halves
            ident = singles.tile([128, 128], BF16, name="ident")
            gamma_sb = singles.tile([128, D], F32, name="gamma_sb")
            beta_sb = singles.tile([128, D], F32, name="beta_sb")
            eps_sb = singles.tile([128, 1], F32, name="eps_sb")

            nc.sync.dma_start(wq_sb, wqT_ext[:].rearrange("(ko p) o -> p ko o", p=128))
            nc.sync.dma_start(wk_sb, wkT_ext[:].rearrange("(ko p) o -> p ko o", p=128))
            nc.sync.dma_start(wv_sb, wvT_ext[:].rearrange("(ko p) o -> p ko o", p=128))
            nc.sync.dma_start(xT_sb, xT_ext[:].rearrange("(ko p) n -> p ko n", p=128))
            make_identity(nc, ident)
            gam_bc = bass.AP(tensor=gamma_ext, offset=0, ap=[[0, 128], [1, D]])
            bet_bc = bass.AP(tensor=beta_ext, offset=0, ap=[[0, 128], [1, D]])
            nc.sync.dma_start(gamma_sb, gam_bc)
            nc.sync.dma_start(beta_sb, bet_bc)
            nc.vector.memset(eps_sb, LN_EPS)

            # ---- DRAM internals ----
            k_shard = dram.tile([NR, D], BF16, name="k_shard")
            v_shard = dram.tile([NR, D], BF16, name="v_shard")
            # gathered K: flat-equal to full K [4096,640]; viewed [10h*64k, 4096keys]
            k_gath = dram.tile([H * DK, N], BF16, name="k_gath", addr_space="Shared")
            # gathered V: flat-equal to full V; viewed [10h*32t, 128p, 64k]
            v_gath = dram.tile([H * 32, 128, DK], BF16, name="v_gath", addr_space="Shared")
            # attention outputs in chunk layout (pre-residual)
            vchunk = dram.tile([R, DK], BF16, name="vchunk")

            # ---- phase 1: K/V projections -> shards -> AllGather ----
            with tc.tile_pool(name="proj_ps", bufs=2, space="PSUM") as pps, \
                 tc.tile_pool(name="proj_sb", bufs=3) as psb:
                for w_sb, shard in ((wk_sb, k_shard), (wv_sb, v_shard)):
                    for nt in range(4):
                        ps_a = pps.tile([128, 512], F32, tag="pp512")
                        ps_b = pps.tile([128, 128], F32, tag="pp128")
                        for ko in range(5):
                            nc.tensor.matmul(ps_a, lhsT=xT_sb[:, ko, bass.ts(nt, 128)],
                                             rhs=w_sb[:, ko, 0:512],
                                             start=(ko == 0), stop=(ko == 4))
                        for ko in range(5):
                            nc.tensor.matmul(ps_b, lhsT=xT_sb[:, ko, bass.ts(nt, 128)],
                                             rhs=w_sb[:, ko, 512:D],
                                             start=(ko == 0), stop=(ko == 4))
                        kv_sb = psb.tile([128, D], BF16, tag="kv_sb")
                        nc.vector.tensor_copy(kv_sb[:, 0:512], ps_a)
                        nc.vector.tensor_copy(kv_sb[:, 512:D], ps_b)
                        nc.sync.dma_start(shard[bass.ts(nt, 128), :], kv_sb)

                nc.gpsimd.collective_compute(
                    "AllGather", mybir.AluOpType.bypass,
                    replica_groups=[list(range(NCORES))],
                    ins=[k_shard.opt()], outs=[k_gath.opt()])
                nc.gpsimd.collective_compute(
                    "AllGather", mybir.AluOpType.bypass,
                    replica_groups=[list(range(NCORES))],
                    ins=[v_shard.opt()], outs=[v_gath.opt()])

                # ---- phase 2: Q projection into transposed chunk layout ----
                # qt3[k, n, j] = Q[n, 64j+k];  qt free index r = 10n+j
                qt3 = qt_sb[:].rearrange("p (n j) -> p n j", j=10)
                for j in range(10):
                    ps_q = pps.tile([64, 512], F32, tag="ppq")
                    for ko in range(5):
                        nc.tensor.matmul(ps_q, lhsT=wq_sb[:, ko, bass.ds(64 * j, 64)],
                                         rhs=xT_sb[:, ko, :],
                                         start=(ko == 0), stop=(ko == 4))
                    nc.vector.tensor_copy(qt3[0:64, :, j], ps_q)
                # duplicate into upper partitions for row-packed S matmuls
                nc.sync.dma_start(qt_sb[64:128, :], qt_sb[0:64, :])

            # ---- phase 3: attention over 5 super-blocks of 1024 chunk rows ----
            gp = nc.gpsimd
            pid = gp.partition_id()

            with tc.tile_pool(name="st_ps", bufs=2, space="PSUM") as st_pool, \
                 tc.tile_pool(name="pv_ps", bufs=1, space="PSUM") as pv_pool, \
                 tc.tile_pool(name="tr_ps", bufs=1, space="PSUM") as tr_pool, \
                 tc.tile_pool(name="kv_io", bufs=2) as kv_io, \
                 tc.tile_pool(name="pt_pool", bufs=2) as pt_pool, \
                 tc.tile_pool(name="att_sb", bufs=4) as att_sb:

                for s5 in range(NSB):
                    h = (pid * NSB + s5) // 4
                    h64 = gp.snap(h * DK, min_val=0, max_val=(H - 1) * DK)
                    h32 = gp.snap(h * 32, min_val=0, max_val=(H - 1) * 32)

                    kb = kv_io.tile([128, 2048], BF16, tag="kb")
                    nc.gpsimd.dma_start(kb[0:64, :], k_gath[bass.ds(h64, 64), 0:2048])
                    nc.gpsimd.dma_start(kb[64:128, :], k_gath[bass.ds(h64, 64), 2048:4096])

                    vb = kv_io.tile([128, 32, DK + 1], BF16, tag="vb")
                    nc.vector.memset(vb[:, :, DK:DK + 1], 1.0)
                    nc.gpsimd.dma_start(
                        vb[:, :, 0:DK],
                        v_gath[bass.ds(h32, 32), :, :].rearrange("t p k -> p t k"))

                    for half in range(2):
                        r0 = 1024 * s5 + 512 * half
                        pt = pt_pool.tile([128, 32, 512], BF16, tag="pt")
                        # S^T matmuls (row-packed pairs) + exp in groups of 3 slots
                        slot = 0
                        for g in range(11):
                            size = 3 if g < 10 else 2
                            st = st_pool.tile([128, 3, 512], F32, tag="st")
                            for u in range(size):
                                s = slot + u
                                hf, col = s % 2, s // 2
                                nc.tensor.matmul(
                                    st[:, u, :],
                                    lhsT=kb[64 * hf:64 * hf + 64, bass.ts(col, 128)],
                                    rhs=qt_sb[64 * hf:64 * hf + 64, bass.ds(r0, 512)],
                                    start=True, stop=True)
                            nc.scalar.activation(
                                pt[:, slot:slot + size, :], st[:, 0:size, :],
                                mybir.ActivationFunctionType.Exp, scale=SCALE)
                            slot += size

                        # P^T @ [V | 1] accumulation -> O^T [65, 512]
                        pv = pv_pool.tile([DK + 1, 512], F32, tag="pv")
                        for s in range(32):
                            mt = 16 * (s % 2) + s // 2
                            nc.tensor.matmul(pv, lhsT=vb[:, mt, :], rhs=pt[:, s, :],
                                             start=(s == 0), stop=(s == 31))
                        ot = att_sb.tile([DK + 1, 512], BF16, tag="ot")
                        nc.vector.tensor_copy(ot, pv)

                        # transpose back to chunk-row layout, divide by denominator
                        for q in range(4):
                            tr = tr_pool.tile([128, DK + 1], BF16, tag="tr")
                            nc.tensor.transpose(tr, ot[:, bass.ts(q, 128)],
                                                ident[0:DK + 1, 0:DK + 1])
                            rec = att_sb.tile([128, 1], F32, tag="rec")
                            nc.vector.reciprocal(rec, tr[:, DK:DK + 1])
                            oc = att_sb.tile([128, DK], BF16, tag="oc")
                            nc.vector.tensor_scalar_mul(oc, tr[:, 0:DK], rec)
                            nc.sync.dma_start(
                                vchunk[bass.ds(r0 + 128 * q, 128), :], oc)

            # ---- phase 4: residual + LayerNorm ----
            vnat = vchunk[:].rearrange("(n j) k -> n (j k)", j=10)  # [512, 640]
            with tc.tile_pool(name="ln_sb", bufs=3) as ln_sb:
                for nt in range(4):
                    vt = ln_sb.tile([128, D], BF16, tag="vt")
                    nc.sync.dma_start(vt, vnat[bass.ts(nt, 128), :])
                    xt = ln_sb.tile([128, D], F32, tag="xt")
                    nc.sync.dma_start(xt, xf_ext[bass.ts(nt, 128), :])
                    val = ln_sb.tile([128, D], F32, tag="val")
                    nc.vector.tensor_add(val, vt, xt)

                    stats = ln_sb.tile([128, 5, 6], F32, tag="stats")
                    for sg in range(5):
                        nc.vector.bn_stats(stats[:, sg, :], val[:, bass.ts(sg, 128)])
                    mv = ln_sb.tile([128, 2], F32, tag="mv")
                    nc.vector.bn_aggr(mv, stats)

                    rstd = ln_sb.tile([128, 1], F32, tag="rstd")
                    nc.scalar.activation(rstd, mv[:, 1:2],
                                         mybir.ActivationFunctionType.Sqrt,
                                         bias=eps_sb)
                    nc.vector.reciprocal(rstd, rstd)

                    nrm = ln_sb.tile([128, D], F32, tag="nrm")
                    nc.vector.tensor_scalar(nrm, val, mv[:, 0:1], rstd,
                                            mybir.AluOpType.subtract,
                                            mybir.AluOpType.mult)
                    nc.vector.tensor_mul(nrm, nrm, gamma_sb)
                    nc.vector.tensor_add(nrm, nrm, beta_sb)
                    nc.sync.dma_start(out_ext[bass.ts(nt, 128), :], nrm)

    nc.compile()
    return nc


_NC = None


def get_nc():
    global _NC
    if _NC is None:
        _NC = build_nc()
    return _NC


def make_in_maps(x, Wq, Wk, Wv, gamma, beta):
    x = np.asarray(x, np.float32)
    wqT = np.ascontiguousarray(np.asarray(Wq, np.float32).T).astype(ml_dtypes.bfloat16)
    wkT = np.ascontiguousarray(np.asarray(Wk, np.float32).T).astype(ml_dtypes.bfloat16)
    wvT = np.ascontiguousarray(np.asarray(Wv, np.float32).T).astype(ml_dtypes.bfloat16)
    gamma = np.asarray(gamma, np.float32)
    beta = np.asarray(beta, np.float32)
    in_maps = []
    for c in range(NCORES):
        xs = x[NR * c:NR * (c + 1)]
        in_maps.append({
            "xT": np.ascontiguousarray(xs.T).astype(ml_dtypes.bfloat16),
            "xf": np.ascontiguousarray(xs),
            "wqT": wqT, "wkT": wkT, "wvT": wvT,
            "gamma": gamma, "beta": beta,
        })
    return in_maps


def kernel(x, Wq, Wk, Wv, gamma, beta):
    nc = get_nc()
    in_maps = make_in_maps(x, Wq, Wk, Wv, gamma, beta)
    res = run_bass_kernel_spmd(nc, in_maps, list(range(NCORES)))
    return np.concatenate([res.results[c]["out"] for c in range(NCORES)], axis=0)


if __name__ == "__main__":
    nc = build_nc()
    print("build+compile OK; instructions:",
          sum(len(bb.instructions) for bb in nc.main_func.blocks))


# revision 3
# speedup vs baseline: 1.0487x; 1.0487x over previous
"""MHA block (10 heads, N=4096, D=640) on 8 trn2 NeuronCores.

Strategy: shard x by rows (512/core). The reference's raw reshapes make head
blocks contiguous in the flat Q/K/V buffers, so core c's query chunk-rows
[5120c, 5120(c+1)) are exactly Q rows [512c, 512(c+1)) and its attention
outputs are exactly output rows [512c, 512(c+1)).

No collectives: each core computes the K/V projections for the 1024-row
window of x that covers its two heads (the host passes that window per core),
writing them to core-local DRAM. Scores are computed transposed (keys on
partitions) so both attention matmuls take natural layouts; keys are
contracted in stride-32 sets {32p + t2} so the V tile loads as one
contiguous-per-partition DMA; the softmax denominator rides along as a
ones-column in the V matmul; exp runs on ACT straight out of PSUM. Per-core
head selection uses partition_id register arithmetic with dynamic-offset DMA.
"""

import sys

sys.path.insert(0, "/opt/trn_rl_repo")

import numpy as np
import ml_dtypes

import concourse.bass as bass
import concourse.mybir as mybir
import concourse.tile as tile
from concourse import bacc
from concourse.bass_utils import run_bass_kernel_spmd
from concourse.masks import make_identity

BF16 = mybir.dt.bfloat16
F32 = mybir.dt.float32

NCORES = 8
N, D = 4096, 640
H, DK = 10, 64
NR = N // NCORES          # 512 x-rows per core
R = NR * (D // DK)        # 5120 chunk rows per core
NSB = 5                   # super-blocks of 1024 chunk rows
KW = 1024                 # per-core K/V projection row window
SCALE = 1.0 / 64.0        # 1/sqrt(N)
LN_EPS = 1e-5


def _rsb(c):
    """128-row-unit start of core c's K/V window."""
    hA = (5 * c) // 4
    a = (262144 * hA) // 81920
    return min(a, 24)


def build_nc():
    nc = bacc.Bacc("TRN2", target_bir_lowering=False, debug=False,
                   num_devices=NCORES)

    xT_ext = nc.dram_tensor("xT", [D, NR], BF16, kind="ExternalInput")
    xkT_ext = nc.dram_tensor("xkT", [D, KW], BF16, kind="ExternalInput")
    xf_ext = nc.dram_tensor("xf", [NR, D], F32, kind="ExternalInput")
    wqT_ext = nc.dram_tensor("wqT", [D, D], BF16, kind="ExternalInput")
    wkT_ext = nc.dram_tensor("wkT", [D, D], BF16, kind="ExternalInput")
    wvT_ext = nc.dram_tensor("wvT", [D, D], BF16, kind="ExternalInput")
    gamma_ext = nc.dram_tensor("gamma", [D], F32, kind="ExternalInput")
    beta_ext = nc.dram_tensor("beta", [D], F32, kind="ExternalInput")
    out_ext = nc.dram_tensor("out", [NR, D], F32, kind="ExternalOutput")

    with tile.TileContext(nc) as tc:
        with tc.tile_pool(name="singles", bufs=1) as singles, \
             tc.tile_pool(name="dram", bufs=1, space="DRAM") as dram:
            # ---- persistent SBUF state ----
            wq_sb = singles.tile([128, 5, D], BF16, name="wq_sb")
            wk_sb = singles.tile([128, 5, D], BF16, name="wk_sb")
            wv_sb = singles.tile([128, 5, D], BF16, name="wv_sb")
            xT_sb = singles.tile([128, 5, NR], BF16, name="xT_sb")
            xkT_sb = singles.tile([128, 5, KW], BF16, name="xkT_sb")
            qt_sb = singles.tile([128, R], BF16, name="qt_sb")  # Q^T chunks, dup halves
            ident = singles.tile([128, 128], BF16, name="ident")
            gamma_sb = singles.tile([128, D], F32, name="gamma_sb")
            beta_sb = singles.tile([128, D], F32, name="beta_sb")
            eps_sb = singles.tile([128, 1], F32, name="eps_sb")

            nc.sync.dma_start(wq_sb, wqT_ext[:].rearrange("(ko p) o -> p ko o", p=128))
            nc.sync.dma_start(wk_sb, wkT_ext[:].rearrange("(ko p) o -> p ko o", p=128))
            nc.sync.dma_start(wv_sb, wvT_ext[:].rearrange("(ko p) o -> p ko o", p=128))
            nc.sync.dma_start(xT_sb, xT_ext[:].rearrange("(ko p) n -> p ko n", p=128))
            nc.sync.dma_start(xkT_sb, xkT_ext[:].rearrange("(ko p) n -> p ko n", p=128))
            make_identity(nc, ident)
            gam_bc = bass.AP(tensor=gamma_ext, offset=0, ap=[[0, 128], [1, D]])
            bet_bc = bass.AP(tensor=beta_ext, offset=0, ap=[[0, 128], [1, D]])
            nc.sync.dma_start(gamma_sb, gam_bc)
            nc.sync.dma_start(beta_sb, bet_bc)
            nc.vector.memset(eps_sb, LN_EPS)

            # ---- core-local DRAM K/V (flat-contiguous row window) ----
            k_loc = dram.tile([160, 4096], BF16, name="k_loc")
            v_loc = dram.tile([320, 2048], BF16, name="v_loc")
            k_locN = k_loc[:].rearrange("r m -> (r m)").rearrange("(n d) -> n d", d=D)
            v_locN = v_loc[:].rearrange("r m -> (r m)").rearrange("(n d) -> n d", d=D)
            vchunk = dram.tile([R, DK], BF16, name="vchunk")

            # ---- phase 1: K/V projections over the per-core row window ----
            with tc.tile_pool(name="proj_ps", bufs=2, space="PSUM") as pps, \
                 tc.tile_pool(name="proj_sb", bufs=3) as psb:
                for w_sb, locN in ((wk_sb, k_locN), (wv_sb, v_locN)):
                    for nt in range(8):
                        ps_a = pps.tile([128, 512], F32, tag="pp512")
                        ps_b = pps.tile([128, 128], F32, tag="pp128")
                        for ko in range(5):
                            nc.tensor.matmul(ps_a, lhsT=xkT_sb[:, ko, bass.ts(nt, 128)],
                                             rhs=w_sb[:, ko, 0:512],
                                             start=(ko == 0), stop=(ko == 4))
                        for ko in range(5):
                            nc.tensor.matmul(ps_b, lhsT=xkT_sb[:, ko, bass.ts(nt, 128)],
                                             rhs=w_sb[:, ko, 512:D],
                                             start=(ko == 0), stop=(ko == 4))
                        kv_sb = psb.tile([128, D], BF16, tag="kv_sb")
                        nc.vector.tensor_copy(kv_sb[:, 0:512], ps_a)
                        nc.vector.tensor_copy(kv_sb[:, 512:D], ps_b)
                        nc.sync.dma_start(locN[bass.ts(nt, 128), :], kv_sb)

                # ---- phase 2: Q projection into transposed chunk layout ----
                # qt3[k, n, j] = Q[n, 64j+k];  qt free index r = 10n+j
                qt3 = qt_sb[:].rearrange("p (n j) -> p n j", j=10)
                for j in range(10):
                    ps_q = pps.tile([64, 512], F32, tag="ppq")
                    for ko in range(5):
                        nc.tensor.matmul(ps_q, lhsT=wq_sb[:, ko, bass.ds(64 * j, 64)],
                                         rhs=xT_sb[:, ko, :],
                                         start=(ko == 0), stop=(ko == 4))
                    nc.vector.tensor_copy(qt3[0:64, :, j], ps_q)
                # duplicate into upper partitions for row-packed S matmuls
                nc.sync.dma_start(qt_sb[64:128, :], qt_sb[0:64, :])

            # ---- phase 3: attention over 5 super-blocks of 1024 chunk rows ----
            gp = nc.gpsimd
            pid = gp.partition_id()
            # window start (128-row units): rsb = min((262144*hA)//81920, 24)
            hA = (pid * NSB) // 4
            a = (262144 * hA) // 81920
            rsb = a - (a + 7) // 32

            with tc.tile_pool(name="st_ps", bufs=2, space="PSUM") as st_pool, \
                 tc.tile_pool(name="pv_ps", bufs=1, space="PSUM") as pv_pool, \
                 tc.tile_pool(name="tr_ps", bufs=1, space="PSUM") as tr_pool, \
                 tc.tile_pool(name="kv_io", bufs=2) as kv_io, \
                 tc.tile_pool(name="pt_pool", bufs=2) as pt_pool, \
                 tc.tile_pool(name="att_sb", bufs=4) as att_sb:

                for s5 in range(NSB):
                    h = (pid * NSB + s5) // 4
                    koff = gp.snap(nc.s_assert_within(
                        h * 64 - rsb * 20, 0, 96, skip_runtime_assert=True))
                    voff = gp.snap(nc.s_assert_within(
                        h * 128 - rsb * 40, 0, 192, skip_runtime_assert=True))

                    kb = kv_io.tile([128, 2048], BF16, tag="kb")
                    nc.gpsimd.dma_start(kb[0:64, :], k_loc[bass.ds(koff, 64), 0:2048])
                    nc.gpsimd.dma_start(kb[64:128, :], k_loc[bass.ds(koff, 64), 2048:4096])
                    # kb3[row, pp, t2] = kb[row, 32*pp + t2]
                    kb3 = kb.rearrange("h (pp t2) -> h pp t2", t2=32)

                    vb = kv_io.tile([128, 32, DK + 1], BF16, tag="vb")
                    nc.vector.memset(vb[:, :, DK:DK + 1], 1.0)
                    nc.gpsimd.dma_start(
                        vb[:, :, 0:DK],
                        v_loc[bass.ds(voff, 128), :].rearrange("p (t k) -> p t k", k=DK))

                    for half in range(2):
                        r0 = 1024 * s5 + 512 * half
                        pt = pt_pool.tile([128, 32, 512], BF16, tag="pt")
                        # S^T matmuls over stride-32 key sets + exp in groups of 3
                        slot = 0
                        for g in range(11):
                            size = 3 if g < 10 else 2
                            st = st_pool.tile([128, 3, 512], F32, tag="st")
                            for u in range(size):
                                t2 = slot + u
                                nc.tensor.matmul(
                                    st[0:64, u, :],
                                    lhsT=kb3[0:64, :, t2],
                                    rhs=qt_sb[0:64, bass.ds(r0, 512)],
                                    start=True, stop=True)
                                nc.tensor.matmul(
                                    st[64:128, u, :],
                                    lhsT=kb3[64:128, :, t2],
                                    rhs=qt_sb[64:128, bass.ds(r0, 512)],
                                    start=True, stop=True)
                            nc.scalar.activation(
                                pt[:, slot:slot + size, :], st[:, 0:size, :],
                                mybir.ActivationFunctionType.Exp, scale=SCALE)
                            slot += size

                        # P^T @ [V | 1] accumulation -> O^T [65, 512]
                        pv = pv_pool.tile([DK + 1, 512], F32, tag="pv")
                        for s in range(32):
                            nc.tensor.matmul(pv, lhsT=vb[:, s, :], rhs=pt[:, s, :],
                                             start=(s == 0), stop=(s == 31))
                        ot = att_sb.tile([DK + 1, 512], BF16, tag="ot")
                        nc.vector.tensor_copy(ot, pv)

                        # transpose back to chunk-row layout, divide by denominator
                        for q in range(4):
                            tr = tr_pool.tile([128, DK + 1], BF16, tag="tr")
                            nc.tensor.transpose(tr, ot[:, bass.ts(q, 128)],
                                                ident[0:DK + 1, 0:DK + 1])
                            rec = att_sb.tile([128, 1], F32, tag="rec")
                            nc.vector.reciprocal(rec, tr[:, DK:DK + 1])
                            oc = att_sb.tile([128, DK], BF16, tag="oc")
                            nc.vector.tensor_scalar_mul(oc, tr[:, 0:DK], rec)
                            nc.sync.dma_start(
                                vchunk[bass.ds(r0 + 128 * q, 128), :], oc)

            # ---- phase 4: residual + LayerNorm ----
            vnat = vchunk[:].rearrange("(n j) k -> n (j k)", j=10)  # [512, 640]
            with tc.tile_pool(name="ln_sb", bufs=3) as ln_sb:
                for nt in range(4):
                    vt = ln_sb.tile([128, D], BF16, tag="vt")
                    nc.sync.dma_start(vt, vnat[bass.ts(nt, 128), :])
                    xt = ln_sb.tile([128, D], F32, tag="xt")
                    nc.sync.dma_start(xt, xf_ext[bass.ts(nt, 128), :])
                    val = ln_sb.tile([128, D], F32, tag="val")
                    nc.vector.tensor_add(val, vt, xt)

                    stats = ln_sb.tile([128, 5, 6], F32, tag="stats")
                    for sg in range(5):
                        nc.vector.bn_stats(stats[:, sg, :], val[:, bass.ts(sg, 128)])
                    mv = ln_sb.tile([128, 2], F32, tag="mv")
                    nc.vector.bn_aggr(mv, stats)

                    rstd = ln_sb.tile([128, 1], F32, tag="rstd")
                    nc.scalar.activation(rstd, mv[:, 1:2],
                                         mybir.ActivationFunctionType.Sqrt,
                                         bias=eps_sb)
                    nc.vector.reciprocal(rstd, rstd)

                    nrm = ln_sb.tile([128, D], F32, tag="nrm")
                    nc.vector.tensor_scalar(nrm, val, mv[:, 0:1], rstd,
                                            mybir.AluOpType.subtract,
                                            mybir.AluOpType.mult)
                    nc.vector.tensor_mul(nrm, nrm, gamma_sb)
                    nc.vector.tensor_add(nrm, nrm, beta_sb)
                    nc.sync.dma_start(out_ext[bass.ts(nt, 128), :], nrm)

    nc.compile()
    return nc


_NC = None


def get_nc():
    global _NC
    if _NC is None:
        _NC = build_nc()
    return _NC


def make_in_maps(x, Wq, Wk, Wv, gamma, beta):
    x = np.asarray(x, np.float32)
    wqT = np.ascontiguousarray(np.asarray(Wq, np.float32).T).astype(ml_dtypes.bfloat16)
    wkT = np.ascontiguousarray(np.asarray(Wk, np.float32).T).astype(ml_dtypes.bfloat16)
    wvT = np.ascontiguousarray(np.asarray(Wv, np.float32).T).astype(ml_dtypes.bfloat16)
    gamma = np.asarray(gamma, np.float32)
    beta = np.asarray(beta, np.float32)
    in_maps = []
    for c in range(NCORES):
        xs = x[NR * c:NR * (c + 1)]
        rs = 128 * _rsb(c)
        xk = x[rs:rs + KW]
        in_maps.append({
            "xT": np.ascontiguousarray(xs.T).astype(ml_dtypes.bfloat16),
            "xkT": np.ascontiguousarray(xk.T).astype(ml_dtypes.bfloat16),
            "xf": np.ascontiguousarray(xs),
            "wqT": wqT, "wkT": wkT, "wvT": wvT,
            "gamma": gamma, "beta": beta,
        })
    return in_maps


def kernel(x, Wq, Wk, Wv, gamma, beta):
    nc = get_nc()
    in_maps = make_in_maps(x, Wq, Wk, Wv, gamma, beta)
    res = run_bass_kernel_spmd(nc, in_maps, list(range(NCORES)))
    return np.concatenate([res.results[c]["out"] for c in range(NCORES)], axis=0)


if __name__ == "__main__":
    nc = build_nc()
    print("build+compile OK")
